# revision 27
# baseline (speedup 1.0000x reference)
"""Global-KNN GCN kernel for Trainium2 (8 NeuronCores, SPMD).

Device computes the 161-GFLOP pairwise score matrix in fp8 (e4m3,
DoubleRow perf mode: 256-deep contraction per matmul at ~1 cyc/col,
2x bf16 FLOP rate), then per-448-column-chunk top-8 (values + indices)
on the DVE over bf16 scores. The centered -0.5*||x_j||^2 ranking term is
folded into the contraction itself: channels 2046/2047 are sacrificed --
the stationary (row) side carries (1, 1) there and the moving (column)
side carries a coarse+residual fp8 split of the centered norm term. All
of x stays SBUF-resident in fp8 (loaded once, ~12.8 MB/core). Each
core's input is rotated so its own 784-row block sits at column 0; the
stationary row panels are slices of the same resident tensor (except the
last channel group, which has its own modified panel).

Host does the cheap O(N*K) part: merges the 14x8 per-chunk candidates,
exact fp32 re-score of the top-64, builds the KNN edge list, and runs the
two small GCN layers (sparse aggregation + dense matmuls).
"""

import os
import sys

import numpy as np

if "/opt/trn_rl_repo" not in sys.path:
    sys.path.insert(0, "/opt/trn_rl_repo")

B, H, W, C = 32, 14, 14, 2048
N = B * H * W            # 6272 nodes
K = 8                    # neighbors (excluding self)
N_CORES = 8
ROWS = N // N_CORES      # 784 rows per core
RT = 112                 # rows per tile
NT = ROWS // RT          # 7 row tiles
NB = 448                 # column chunk (psum free size)
NCH = N // NB            # 14 column chunks
KP = C // 256            # 8 channel pair-chunks (256 channels each)
CAND = NCH * 8           # 112 candidates per row
TOPC = 64                # host exact re-score depth

LAST_EXEC_NS = None
LAST_KNN = None
_PROG = None


def _build_program():
    from concourse import bacc, tile, mybir

    f32 = mybir.dt.float32
    bf16 = mybir.dt.bfloat16
    f8 = mybir.dt.float8e4
    u16 = mybir.dt.uint16

    nc = bacc.Bacc("TRN2", target_bir_lowering=False)
    x8 = nc.declare_dram_parameter("x8", [KP, 128, 2, N], f8, isOutput=False)
    xr7d = nc.declare_dram_parameter("xr7", [128, 2, ROWS], f8, isOutput=False)
    cand = nc.declare_dram_parameter("cand", [NT, 2, RT, 112], u16, isOutput=True)

    Act = mybir.ActivationFunctionType
    DR = mybir.MatmulPerfMode.DoubleRow

    with tile.TileContext(nc) as tc:
        with (
            tc.tile_pool(name="persist", bufs=1) as pp,
            tc.tile_pool(name="score", bufs=6) as cp,
            tc.tile_pool(name="stage", bufs=4) as sp,
            tc.tile_pool(name="psum", bufs=8, space="PSUM") as psp,
        ):
            xs = [pp.tile([128, 2, N], f8, name=f"xs{kp}") for kp in range(KP)]
            xr7 = pp.tile([128, 2, ROWS], f8)
            HALF = N // 2

            # all loads on the single sync HW-DGE queue: a second concurrent
            # DMA stream into SBUF slows every matmul ~20% (SBUF write
            # contention with the PE's weight/moving fetch)
            def load(kp, c0, c1):
                nc.sync.dma_start(
                    out=xs[kp][:, :, c0:c1],
                    in_=x8[kp, :, :, c0:c1])
            for kp in range(3):
                load(kp, 0, HALF)
            nc.sync.dma_start(out=xr7[:], in_=xr7d[:])
            for kp in range(3, KP):
                load(kp, 0, HALF)
            for kp in range(KP):
                load(kp, HALF, N)

            for jb in range(2):
                for t in range(NT):
                    r0 = t * RT
                    pss = [
                        psp.tile([RT, NB], f32, tag="ps", name=f"ps_{t}_{jb}_{jp}")
                        for jp in range(7)
                    ]
                    def mm(kp, jp):
                        lhsT = (xs[kp][:, :, r0:r0 + RT] if kp < KP - 1
                                else xr7[:, :, r0:r0 + RT])
                        j = jb * 7 + jp
                        nc.tensor.matmul(
                            pss[jp][:, :],
                            lhsT,
                            xs[kp][:, :, j * NB:(j + 1) * NB],
                            start=(kp == 0), stop=(kp == KP - 1),
                            perf_mode=DR, skip_group_check=True,
                        )
                    if jb == 0 and t < 2:
                        # kp-outer while the x8 halves are still streaming in
                        for kp in range(KP):
                            for jp in range(7):
                                mm(kp, jp)
                    else:
                        # jp-outer: each psum tile completes early in the
                        # block so its top-k drain overlaps the matmuls
                        for jp in range(7):
                            for kp in range(KP):
                                mm(kp, jp)
                    stage = sp.tile([RT, 112], u16, tag="st")
                    for jp in range(7):
                        cb = cp.tile([RT, NB], bf16, tag="cb")
                        nc.scalar.activation(cb[:, :], pss[jp][:, :], Act.Copy)
                        o0 = jp * 16
                        nc.vector.max(stage[:, o0:o0 + 8].bitcast(bf16), cb[:, :])
                        nc.vector.max_index(
                            stage[:, o0 + 8:o0 + 16],
                            stage[:, o0:o0 + 8].bitcast(bf16),
                            cb[:, :])
                    nc.sync.dma_start(out=cand[t, jb], in_=stage[:, :])
    nc.compile()
    return nc


def _knn_from_device(x_flat):
    """Run the SPMD program; return knn [N, K] int64 global indices."""
    global LAST_EXEC_NS, LAST_KNN, _PROG
    import ml_dtypes
    from concourse.bass_utils import run_bass_kernel_spmd

    if _PROG is None:
        _PROG = _build_program()

    xq8 = x_flat.astype(ml_dtypes.float8_e4m3)               # [N, C]
    sq = np.sum(x_flat * x_flat, axis=1, dtype=np.float32)
    nhc = -0.5 * (sq - sq.mean())
    a = nhc.astype(ml_dtypes.float8_e4m3)
    bres = (nhc - a.astype(np.float32)).astype(ml_dtypes.float8_e4m3)
    # x8 layout [kp, p, i, n]: channel = kp*256 + i*128 + p
    x8T = np.ascontiguousarray(xq8.T)                        # [C, N]
    x8 = np.ascontiguousarray(
        x8T.reshape(KP, 2, 128, N).transpose(0, 2, 1, 3))    # [kp, p, i, n]
    # fold the norm term into sacrificed channels 2046/2047 (kp=7, i=1,
    # p=126/127): moving side carries (a, b); stationary side carries (1, 1)
    x8[KP - 1, 126, 1, :] = a
    x8[KP - 1, 127, 1, :] = bres

    one8 = np.float32(1.0).astype(ml_dtypes.float8_e4m3)
    in_maps = []
    for c in range(N_CORES):
        sh = c * ROWS
        x8c = np.ascontiguousarray(np.roll(x8, -sh, axis=3))
        xr7 = np.ascontiguousarray(x8c[KP - 1, :, :, 0:ROWS])
        xr7[126, 1, :] = one8
        xr7[127, 1, :] = one8
        in_maps.append({"x8": x8c, "xr7": xr7})
    res = run_bass_kernel_spmd(
        _PROG, in_maps, list(range(N_CORES)),
        trace=bool(os.environ.get("KNN_TRACE")),
    )
    if res.exec_time_ns is not None:
        LAST_EXEC_NS = res.exec_time_ns

    vals = np.empty((N, CAND), dtype=np.float32)
    cidx = np.empty((N, CAND), dtype=np.int64)
    jbase = (np.arange(NCH, dtype=np.int64) * NB)[None, :, None]
    for c, r in enumerate(res.results):
        o = r["cand"].reshape(NT, 2, RT, 7, 16)
        o = o.transpose(0, 2, 1, 3, 4).reshape(ROWS, NCH, 16)
        v = np.ascontiguousarray(o[:, :, 0:8]).view(ml_dtypes.bfloat16)
        loc = o[:, :, 8:16].astype(np.int64)
        gcol = (jbase + loc + c * ROWS) % N
        vals[c * ROWS:(c + 1) * ROWS] = v.astype(np.float32).reshape(ROWS, CAND)
        cidx[c * ROWS:(c + 1) * ROWS] = gcol.reshape(ROWS, CAND)

    # coarse top-TOPC by device score, then exact fp32 re-score
    part = np.argpartition(-vals, TOPC, axis=1)[:, :TOPC]
    cidx = np.take_along_axis(cidx, part, axis=1)            # [N, TOPC]
    exact = np.empty((N, TOPC), dtype=np.float32)
    BLK = 196
    for r0 in range(0, N, BLK):
        r1 = r0 + BLK
        cn = cidx[r0:r1]                                     # [b, TOPC]
        xc = x_flat[cn]                                      # [b, TOPC, C]
        exact[r0:r1] = np.einsum("bc,bkc->bk", x_flat[r0:r1], xc,
                                 dtype=np.float32) - 0.5 * sq[cn]
    order = np.argsort(-exact, axis=1, kind="stable")[:, :K + 1]
    top = np.take_along_axis(cidx, order, axis=1)            # [N, 9]
    rows = np.arange(N)[:, None]
    selfpos = top == rows
    has_self = selfpos.any(axis=1)
    rem = np.where(has_self, selfpos.argmax(axis=1), K)      # drop self, else 9th
    keep = np.ones((N, K + 1), dtype=bool)
    keep[np.arange(N), rem] = False
    LAST_KNN = top[keep].reshape(N, K)
    return LAST_KNN


def kernel(x, W1, b1, W2, b2):
    x = np.asarray(x, dtype=np.float32)
    W1 = np.asarray(W1, dtype=np.float32)
    b1 = np.asarray(b1, dtype=np.float32)
    W2 = np.asarray(W2, dtype=np.float32)
    b2 = np.asarray(b2, dtype=np.float32)

    xf = x.reshape(N, C)
    knn = _knn_from_device(xf)

    src = np.repeat(np.arange(N, dtype=np.int64), K)
    dst = knn.reshape(-1)
    loops = np.arange(N, dtype=np.int64)
    src = np.concatenate([src, loops])
    dst = np.concatenate([dst, loops])

    deg = np.bincount(dst, minlength=N).astype(np.float32)
    dinv = 1.0 / np.sqrt(np.maximum(deg, 1.0))
    norm = (dinv[src] * dinv[dst]).astype(np.float32)

    try:
        import scipy.sparse as sps
        A = sps.csr_matrix((norm, (dst, src)), shape=(N, N), dtype=np.float32)

        def agg(hw):
            return A @ hw
    except Exception:
        def agg(hw):
            out = np.zeros_like(hw)
            np.add.at(out, dst, hw[src] * norm[:, None])
            return out

    h1 = np.maximum(agg(xf @ W1) + b1, 0.0).astype(np.float32)
    h2 = np.maximum(agg(h1 @ W2) + b2, 0.0).astype(np.float32)
    return h2.reshape(B, H, W, W2.shape[1]).astype(np.float32)
